# revision 4
# baseline (speedup 1.0000x reference)
"""LSTM layer kernel for Trainium2 (8 NeuronCores) — data-parallel over batch.

Each core owns 4 batch rows and the FULL weights (no collectives).

Phase 1 (projection): xg = x_j @ W_ih.T + bias in two gate-halves of 2048
(W_ih half resident 8MB at a time), written to DRAM [S, 4, 4096].

Phase 2 (scan): W_hh.T fully resident in SBUF (16MB). Per timestep:
  - two PSUM halves [4, 2048]: 32 accumulating matmuls each (K-contiguous)
  - xg_t added into PSUM via VectorE, sigmoid([i|f]) overlaps half-B matmuls
  - sigmoid(o), tanh(g), c/h update on [4, 1024] tiles
  - h [4,1024] -> 8 PE transposes -> h.T [128, 32] for the next step's lhsT

Gate row order is [i | f | o | g] (reordered on host).
"""

import numpy as np

import concourse.bass as bass
import concourse.bacc as bacc
import concourse.mybir as mybir
from concourse import tile
from concourse.bass_utils import run_bass_kernel_spmd

F32 = mybir.dt.float32
F32R = mybir.dt.float32r
AF = mybir.ActivationFunctionType


def _r(ap):
    return ap.bitcast(F32R)

B = 32          # full batch
I = 1024
H = 1024
G = 4 * H       # 4096 gates
NCORES = 8
BC = B // NCORES        # 4 batch rows per core
KI = I // 128           # 8
KH = H // 128           # 8
GH = G // 2             # 2048 per half

_CACHE = {}


def build_nc(S):
    nc = bacc.Bacc(
        "TRN2", target_bir_lowering=False, debug=False, num_devices=NCORES
    )
    BT = BC * S             # 2048 bt rows per core
    MT = BT // 128          # 16 projection M-tiles (t-major: tile m = t in [32m,32m+32))

    xTj = nc.dram_tensor("xTj", [I, BT], F32, kind="ExternalInput").ap()
    wihT = nc.dram_tensor("wihT", [I, G], F32, kind="ExternalInput").ap()
    whhT = nc.dram_tensor("whhT", [H, G], F32, kind="ExternalInput").ap()
    bias_bc = nc.dram_tensor("bias_bc", [128, G], F32, kind="ExternalInput").ap()
    h0Tj = nc.dram_tensor("h0Tj", [H, BC], F32, kind="ExternalInput").ap()
    c0j = nc.dram_tensor("c0j", [BC, H], F32, kind="ExternalInput").ap()
    eye4 = nc.dram_tensor("eye4", [BC, BC], F32, kind="ExternalInput").ap()

    outj = nc.dram_tensor("outj", [BC, S, H], F32, kind="ExternalOutput").ap()
    clastj = nc.dram_tensor("clastj", [BC, H], F32, kind="ExternalOutput").ap()

    xg = nc.dram_tensor("xg", [S, BC, G], F32).ap()  # internal scratch

    with tile.TileContext(nc) as tc:
        # ---------------- phase 1: input projection ----------------
        with (
            tc.tile_pool(name="pconst", bufs=1) as pconstp,
            tc.tile_pool(name="pwih", bufs=1) as pwihp,
            tc.tile_pool(name="plhs", bufs=3) as plhsp,
            tc.tile_pool(name="ppsum", bufs=2, space="PSUM") as ppsump,
            tc.tile_pool(name="pout", bufs=3) as poutp,
        ):
            bias_sb = pconstp.tile([128, G], F32)
            nc.sync.dma_start(bias_sb, bias_bc)
            for half in range(2):
                wih_sb = pwihp.tile([128, KI * GH], F32, tag="wih")
                nc.sync.dma_start(
                    wih_sb.rearrange("p (k g) -> p k g", k=KI),
                    wihT[:, GH * half : GH * (half + 1)].rearrange(
                        "(k p) g -> p k g", p=128
                    ),
                )
                for m in range(MT):
                    lhs = plhsp.tile([128, KI * 128], F32, tag="plhs")
                    nc.sync.dma_start(
                        lhs.rearrange("p (k c) -> p k c", k=KI),
                        xTj[:, 128 * m : 128 * (m + 1)].rearrange(
                            "(k p) c -> p k c", p=128
                        ),
                    )
                    ps = ppsump.tile([128, GH], F32, tag="ppsum")
                    for n in range(GH // 512):
                        for k in range(KI):
                            nc.tensor.matmul(
                                ps[:, 512 * n : 512 * (n + 1)],
                                lhs[:, 128 * k : 128 * (k + 1)],
                                wih_sb[:, GH * k + 512 * n : GH * k + 512 * (n + 1)],
                                start=(k == 0),
                                stop=(k == KI - 1),
                            )
                    ob = poutp.tile([128, GH], F32, tag="pout")
                    nc.vector.tensor_add(
                        ob, ps, bias_sb[:, GH * half : GH * (half + 1)]
                    )
                    nc.sync.dma_start(
                        xg[32 * m : 32 * (m + 1), :, GH * half : GH * (half + 1)]
                        .rearrange("t b g -> (t b) g"),
                        ob,
                    )

        # ---------------- phase 2: recurrent scan ----------------
        with (
            tc.tile_pool(name="sconst", bufs=1) as sconstp,
            tc.tile_pool(name="xgp", bufs=1) as xgp,
            tc.tile_pool(name="spsum", bufs=1, space="PSUM") as spsump,
            tc.tile_pool(name="hTp", bufs=2) as hTp,
            tc.tile_pool(name="state", bufs=2) as statep,
            tc.tile_pool(name="act", bufs=1) as actp,
        ):
            eye_sb = sconstp.tile([BC, BC], F32)
            nc.sync.dma_start(eye_sb, eye4)
            whh_sb = sconstp.tile([128, KH * G], F32)  # 16MB resident
            nc.sync.dma_start(
                whh_sb.rearrange("p (k g) -> p k g", k=KH),
                whhT.rearrange("(k p) g -> p k g", p=128),
            )

            hT_sb = hTp.tile([128, KH * BC], F32, tag="hT")  # h.T; k-tile = [:, 4k:4k+4]
            nc.sync.dma_start(
                hT_sb.rearrange("p (k b) -> p k b", k=KH),
                h0Tj.rearrange("(k p) b -> p k b", p=128),
            )
            c_old = statep.tile([BC, H], F32, tag="c")
            nc.sync.dma_start(c_old, c0j)

            for t in range(S):
                xg_t = xgp.tile([BC, G], F32, tag="xg")
                nc.sync.dma_start(xg_t, xg[t])

                # half A: gates [i|f] ; half B: gates [o|g]
                psA = spsump.tile([BC, GH], F32, tag="gA")
                psB = spsump.tile([BC, GH], F32, tag="gB")
                for half, ps in ((0, psA), (1, psB)):
                    for n in range(GH // 512):
                        for k in range(KH):
                            nc.tensor.matmul(
                                ps[:, 512 * n : 512 * (n + 1)],
                                hT_sb[:, BC * k : BC * (k + 1)],
                                whh_sb[
                                    :,
                                    G * k + GH * half + 512 * n :
                                    G * k + GH * half + 512 * (n + 1),
                                ],
                                start=(k == 0),
                                stop=(k == KH - 1),
                            )
                    # add xg into PSUM (VectorE; overlaps the other half's MMs)
                    nc.vector.tensor_add(
                        ps, ps, xg_t[:, GH * half : GH * (half + 1)]
                    )

                sig_if = actp.tile([BC, GH], F32)
                nc.scalar.activation(sig_if, psA, AF.Sigmoid)
                sig_o = actp.tile([BC, H], F32)
                nc.scalar.activation(sig_o, psB[:, 0:H], AF.Sigmoid)
                tg = actp.tile([BC, H], F32)
                nc.scalar.activation(tg, psB[:, H:GH], AF.Tanh)

                ig = actp.tile([BC, H], F32)
                nc.vector.tensor_mul(ig, sig_if[:, 0:H], tg)
                cf = actp.tile([BC, H], F32)
                nc.vector.tensor_mul(cf, sig_if[:, H:GH], c_old)
                c_new = statep.tile([BC, H], F32, tag="c")
                nc.vector.tensor_add(c_new, ig, cf)

                tch = actp.tile([BC, H], F32)
                nc.scalar.activation(tch, c_new, AF.Tanh)
                h_b = actp.tile([BC, H], F32, bufs=2)
                nc.vector.tensor_mul(h_b, sig_o, tch)

                nc.sync.dma_start(outj[:, t, :], h_b)

                # h [4,1024] -> h.T [128, 32] via 8 PE transposes (tag shares gA slot)
                psT = spsump.tile([128, KH * BC], F32, tag="gA")
                for k in range(KH):
                    nc.tensor.transpose(
                        psT[:, BC * k : BC * (k + 1)],
                        h_b[:, 128 * k : 128 * (k + 1)],
                        eye_sb,
                    )
                hT_sb = hTp.tile([128, KH * BC], F32, tag="hT")
                nc.vector.tensor_copy(hT_sb, psT)
                c_old = c_new

            nc.sync.dma_start(clastj, c_old)

    nc.compile()
    return nc


def _prep_inputs(input_seq, h0, c0, weight_ih, bias_ih, weight_hh, bias_hh):
    S = input_seq.shape[1]
    f32 = np.float32
    input_seq = np.asarray(input_seq, dtype=f32)
    h0 = np.asarray(h0, dtype=f32)
    c0 = np.asarray(c0, dtype=f32)

    def reorder_rows(w):
        # [4H, ...] in (i, f, g, o) blocks -> [i | f | o | g]
        wi, wf, wg, wo = np.split(np.asarray(w, dtype=f32), 4, axis=0)
        return np.concatenate([wi, wf, wo, wg], axis=0)

    wihT = np.ascontiguousarray(reorder_rows(weight_ih).T)
    whhT = np.ascontiguousarray(reorder_rows(weight_hh).T)
    bias = reorder_rows((np.asarray(bias_ih) + np.asarray(bias_hh))[:, None]).reshape(G)
    bias_bc = np.ascontiguousarray(np.broadcast_to(bias[None, :], (128, G)))
    eye = np.eye(BC, dtype=f32)

    in_maps = []
    for j in range(NCORES):
        bs = slice(j * BC, (j + 1) * BC)
        in_maps.append(
            {
                "xTj": np.ascontiguousarray(
                    input_seq[bs].transpose(2, 1, 0).reshape(I, S * BC)
                ),
                "wihT": wihT,
                "whhT": whhT,
                "bias_bc": bias_bc,
                "h0Tj": np.ascontiguousarray(h0[bs].T),
                "c0j": np.ascontiguousarray(c0[bs]),
                "eye4": eye,
            }
        )
    return in_maps


def kernel(input_seq, h0, c0, weight_ih, bias_ih, weight_hh, bias_hh):
    S = input_seq.shape[1]
    if S not in _CACHE:
        _CACHE[S] = build_nc(S)
    nc = _CACHE[S]

    in_maps = _prep_inputs(
        input_seq, h0, c0, weight_ih, bias_ih, weight_hh, bias_hh
    )
    res = run_bass_kernel_spmd(nc, in_maps, list(range(NCORES)))

    output_seq = np.concatenate(
        [res.results[j]["outj"] for j in range(NCORES)], axis=0
    ).astype(np.float32)
    c_last = np.concatenate(
        [res.results[j]["clastj"] for j in range(NCORES)], axis=0
    ).astype(np.float32)
    h_last = np.ascontiguousarray(output_seq[:, -1, :])
    return output_seq, h_last, c_last
